# revision 20
# baseline (speedup 1.0000x reference)
"""Multi-head causal attention (B=2, S=2048, D=1024, H=16) on 8 TRN2 cores.

Sharding: core = (batch b = core//4, head-group g = core%4). Each core
computes 4 heads of one batch end-to-end (QKV projections for its head
slice, causal attention, its partial contribution to the output
projection). Host sums the 4 partial outputs per batch and adds the bias.

Device algorithm (per core), all matmuls in bf16 with f32 PSUM accum.

Data movement: x inputs and weights are host-prearranged into the exact
per-partition SBUF image so every DMA is 128 rows of >=4KB contiguous
bytes (descriptor-generation cost scales with descriptor count). All
input DMAs are dependency-free and issue up front: q/k chunk slabs on
the sync HWDGE ring, v on the gpsimd SWDGE ring, weights on the scalar
ring. Output partials are staged to bf16 full 2KB rows.

Compute schedule: qT/kT [dloc=256, S] = Wslice @ x.T (N=512 streams at
full rate). V [s, dloc] matmuls (N=256, stationary = x slices) would be
LDWEIGHTS-bound on their own, so each one is woven between N=512
matmuls (q/k projection or Wo) whose streams hide the weight loads.
Attention runs per (q-chunk of 512, head PAIR): the two heads' K=64
score matmuls are row-tiled into PE halves (concurrent); exp via one
strided ScalarE op per k-tile pair; only the 128x128 block ON the
diagonal gets the 0/1 mask multiply; AV for both heads is emitted
inline per k-tile (V_aug carries a ones column -> PSUM row 64 is the
softmax denominator). ScalarE exp is the per-tile bottleneck during
attention, so independent PE work is woven in at k-tile granularity:
chunk c's attention carries the chunk-c+1 q/k sweeps (plus deferred V
matmuls), and the Wo projections are deferred to the exp-heaviest last
chunk. Host sums the 4 bf16 partials per batch and adds the bias.

The device kernel assumes the causal (lower-triangular) mask the
reference constructs; kernel() verifies that and falls back to an exact
numpy implementation for any other mask.
"""

import numpy as np
import ml_dtypes

D_MODEL = 1024
NUM_HEADS = 16
HEAD_DIM = 64
B = 2
S = 2048
N_CORES = 8
GROUPS = 4                 # head-groups (cores per batch)
HPC = NUM_HEADS // GROUPS  # 4 heads per core
DLOC = HPC * HEAD_DIM      # 256 local projection dims
P = 128
SCH = 512                  # q/s chunk
NCH = S // SCH             # 4
KT = S // P                # 16 k-tiles
IT = D_MODEL // P          # 8 contraction tiles
MB = DLOC // P             # 2 m-blocks

_CACHE = {}


def _build():
    import concourse.bass as bass
    import concourse.tile as tile
    from concourse import bacc, mybir

    F32 = mybir.dt.float32
    BF16 = mybir.dt.bfloat16

    nc = bacc.Bacc("TRN2", target_bir_lowering=False, debug=False,
                   num_devices=N_CORES)

    xq = nc.dram_tensor("xq_t", [P, NCH, IT, SCH], BF16, kind="ExternalInput")
    xk = nc.dram_tensor("xk_t", [P, NCH, IT, SCH], BF16, kind="ExternalInput")
    xv = nc.dram_tensor("xv_t", [P, NCH, IT, SCH], BF16, kind="ExternalInput")
    wq = nc.dram_tensor("wq_t", [P, IT, DLOC], BF16, kind="ExternalInput")
    wk = nc.dram_tensor("wk_t", [P, IT, DLOC], BF16, kind="ExternalInput")
    wv = nc.dram_tensor("wv_t", [P, IT, DLOC], BF16, kind="ExternalInput")
    wo = nc.dram_tensor("wo_t", [P, MB, D_MODEL], BF16, kind="ExternalInput")
    mk = nc.dram_tensor("mask", [P, P], BF16, kind="ExternalInput")
    outp = nc.dram_tensor("outp", [S, D_MODEL], BF16, kind="ExternalOutput")

    Exp = mybir.ActivationFunctionType.Exp

    with tile.TileContext(nc) as tc:
        with (
            tc.tile_pool(name="const", bufs=1) as constp,
            tc.tile_pool(name="persist", bufs=1) as pers,
            tc.tile_pool(name="attn", bufs=8) as attnp,
            tc.tile_pool(name="small", bufs=4) as small,
            tc.tile_pool(name="ostage", bufs=5) as ostage,
            tc.tile_pool(name="psA", bufs=2, space="PSUM") as psA,
            tc.tile_pool(name="psS", bufs=2, space="PSUM") as psS,
            tc.tile_pool(name="psO", bufs=2, space="PSUM") as psO,
        ):
            # ---- constants / persistent tensors ----
            wq_sb = constp.tile([P, IT, DLOC], BF16)
            wk_sb = constp.tile([P, IT, DLOC], BF16)
            wv_sb = constp.tile([P, IT, DLOC], BF16)
            wo_sb = constp.tile([P, MB, D_MODEL], BF16)
            mk_sb = constp.tile([P, P], BF16)

            xq_sb = pers.tile([P, NCH, IT, SCH], BF16)
            xk_sb = pers.tile([P, NCH, IT, SCH], BF16)
            xv_sb = pers.tile([P, NCH, IT, SCH], BF16)

            qT_sb = pers.tile([P, MB, S], BF16)
            kT_sb = pers.tile([P, MB, S], BF16)
            v_sb = pers.tile([P, KT, HPC, HEAD_DIM + 1], BF16)
            atn_sb = pers.tile([P, MB, S], BF16)

            # weights on the scalar ring, in first-use order
            nc.scalar.dma_start(wq_sb[:], wq[:])
            nc.scalar.dma_start(wk_sb[:], wk[:])
            nc.scalar.dma_start(wv_sb[:], wv[:])
            nc.scalar.dma_start(mk_sb[:], mk[:])
            nc.scalar.dma_start(wo_sb[:], wo[:])

            # q/k chunk slabs on the sync ring, v on the gpsimd ring;
            # dependency-free, so they all issue immediately.
            for c in range(NCH):
                nc.sync.dma_start(xq_sb[:, c], xq[:, c])
                nc.sync.dma_start(xk_sb[:, c], xk[:, c])
            for c in range(NCH):
                nc.gpsimd.dma_start(xv_sb[:, c], xv[:, c])

            nc.vector.memset(v_sb[:, :, :, HEAD_DIM:HEAD_DIM + 1], 1.0)

            # ---- filler machinery: lists of single-step closures ----

            def v_pulls(vc, j, pool):
                # V projection for s-tile 4*vc+j as 8 r-step pulls, each
                # one N=256 matmul meant to follow an N=512 stream that
                # hides its LDWEIGHTS.
                st = {}

                def pull(r):
                    if r == 0:
                        st["ps"] = pool.tile([P, DLOC], F32, tag=pool.name,
                                             name="psv")
                    nc.tensor.matmul(
                        st["ps"][:], xv_sb[:, vc, r, j * P:(j + 1) * P],
                        wv_sb[:, r, :],
                        start=(r == 0), stop=(r == IT - 1))
                    if r == IT - 1:
                        nc.vector.tensor_copy(
                            v_sb[:, 4 * vc + j, :, 0:HEAD_DIM],
                            st["ps"][:].rearrange("p (h d) -> p h d", h=HPC))

                return [lambda r=r: pull(r) for r in range(IT)]

            def qk_steps(c, part, m, weave=()):
                # one N=512 q/k projection sweep; `weave` holds v-pull
                # iterators advanced one pull per step
                x_sb, w_sb, dst = ((xq_sb, wq_sb, qT_sb) if part == 0 else
                                   (xk_sb, wk_sb, kT_sb))
                st = {}

                def step(r):
                    if r == 0:
                        st["ps"] = psA.tile([P, SCH], F32, tag="psA",
                                            name="psqk")  # noqa
                    nc.tensor.matmul(
                        st["ps"][:], w_sb[:, r, m * P:(m + 1) * P],
                        x_sb[:, c, r, :],
                        start=(r == 0), stop=(r == IT - 1))
                    for w in weave:
                        if w:
                            w.pop(0)()
                    if r == IT - 1:
                        nc.vector.tensor_copy(
                            dst[:, m, c * SCH:(c + 1) * SCH], st["ps"][:])

                return [lambda r=r: step(r) for r in range(IT)]

            def wo_steps(c, weave=()):
                # Wo projection for chunk c as single-matmul steps; each
                # step may also advance one woven v pull
                steps = []
                for t in [4 * c + i for i in range(4)]:
                    st = {}

                    def step(t, oc, m, st=None):
                        if oc == 0 and m == 0:
                            st["ot"] = ostage.tile([P, D_MODEL], BF16,
                                                   tag="ot", name="ot")
                        if m == 0:
                            st["ps"] = psA.tile([P, SCH], F32, tag="psA",
                                                name="pso")
                        nc.tensor.matmul(
                            st["ps"][:], atn_sb[:, m, t * P:(t + 1) * P],
                            wo_sb[:, m, oc * SCH:(oc + 1) * SCH],
                            start=(m == 0), stop=(m == MB - 1))
                        for w in weave:
                            if w:
                                w.pop(0)()
                        if m == MB - 1:
                            nc.vector.tensor_copy(
                                st["ot"][:, oc * SCH:(oc + 1) * SCH],
                                st["ps"][:])
                            if oc == D_MODEL // SCH - 1:
                                nc.gpsimd.dma_start(
                                    outp[t * P:(t + 1) * P, :], st["ot"][:])

                    for oc in range(D_MODEL // SCH):
                        for m in range(MB):
                            steps.append(
                                lambda t=t, oc=oc, m=m, st=st:
                                step(t, oc, m, st))
                return steps

            def norm_head(h, c, ps_at):
                # AT[0:64] *= broadcast(1/l);  l = ps_at row 64.
                # approx_fast mishandles partition-offset inputs: stage the
                # l row to partition 0 first.
                m, po = h // 2, (h % 2) * HEAD_DIM
                lrow = small.tile([1, SCH], F32, tag="lrow")
                nc.vector.tensor_copy(lrow[:], ps_at[HEAD_DIM:HEAD_DIM + 1, :])
                linv = small.tile([1, SCH], F32, tag="linv")
                nc.vector.reciprocal_approx_fast(out=linv[:], in_=lrow[:])
                lbc = small.tile([HEAD_DIM, SCH], F32, tag="lbc")
                nc.gpsimd.partition_broadcast(lbc[:], linv[:])
                nc.vector.tensor_mul(
                    atn_sb[po:po + HEAD_DIM, m, c * SCH:(c + 1) * SCH],
                    ps_at[0:HEAD_DIM, :], lbc[:])

            # ---- prologue: chunk-0 projections ----
            # q sweeps depend only on wq+xq[0] (first DMAs to land); the k
            # sweeps carry chunk-0's V matmuls two pulls per step (the
            # second borrows a psS slot, idle until attention starts).
            for m in range(MB):
                for f in qk_steps(0, 0, m):
                    f()
            for m in range(MB):
                vw = [v_pulls(0, 2 * m, psA), v_pulls(0, 2 * m + 1, psS)]
                for f in qk_steps(0, 1, m, weave=vw):
                    f()

            # per-chunk filler step lists (consumed at k-tile granularity
            # inside the attention loops). v(c) must land before chunk c's
            # AV chain reaches its k-tiles, so v(c+1) rides chunk c.
            filler = {
                0: (qk_steps(1, 0, 0, [v_pulls(1, 0, psA)]) +
                    qk_steps(1, 0, 1, [v_pulls(1, 1, psA)]) +
                    qk_steps(1, 1, 0, [v_pulls(1, 2, psA)]) +
                    qk_steps(1, 1, 1, [v_pulls(1, 3, psA)])),
                1: (qk_steps(2, 0, 0, [v_pulls(2, 0, psA)]) +
                    qk_steps(2, 0, 1, [v_pulls(2, 1, psA)]) +
                    qk_steps(2, 1, 0, [v_pulls(2, 2, psA)]) +
                    qk_steps(2, 1, 1, [v_pulls(2, 3, psA)])),
                2: (qk_steps(3, 0, 0, [v_pulls(3, 0, psA)]) +
                    qk_steps(3, 0, 1, [v_pulls(3, 1, psA)]) +
                    qk_steps(3, 1, 0, [v_pulls(3, 2, psA)]) +
                    qk_steps(3, 1, 1, [v_pulls(3, 3, psA)]) +
                    wo_steps(0)),
                3: wo_steps(1) + wo_steps(2),
            }

            # ---- attention: chunk-major, head pairs, AV inline per k-tile
            for c in range(NCH):
                nkt = 4 * (c + 1)  # causal: k-tiles 0..nkt-1
                steps = filler[c]
                nslots = 2 * 2 * nkt
                acc = 0.0
                rate = len(steps) / nslots

                def qoff(kt):
                    # diagonal k-tile j only needs q in [128j, 512)
                    return max(kt - 4 * c, 0) * P

                for hp in range(HPC // 2):
                    ps_at = [psO.tile([HEAD_DIM + 1, SCH], F32, tag="psO",
                                      name="ps_at") for _ in range(2)]
                    for kt in range(nkt):
                        qo = qoff(kt)
                        ps_s = psS.tile([P, 2, SCH], F32, tag="psS")
                        for hh in range(2):
                            h = 2 * hp + hh
                            m, po = h // 2, (h % 2) * HEAD_DIM
                            nc.tensor.matmul(
                                ps_s[:, hh, qo:],
                                kT_sb[po:po + HEAD_DIM, m,
                                      kt * P:(kt + 1) * P],
                                qT_sb[po:po + HEAD_DIM, m,
                                      c * SCH + qo:(c + 1) * SCH],
                                start=True, stop=True)
                        att = attnp.tile([P, 2, SCH], BF16, tag="attn")
                        nc.scalar.activation(att[:, :, qo:],
                                             ps_s[:, :, qo:], Exp, scale=0.125)
                        if kt - 4 * c >= 0:
                            # only the 128x128 block on the diagonal needs
                            # the triangular mask
                            for hh in range(2):
                                nc.vector.tensor_mul(
                                    att[:, hh, qo:qo + P],
                                    att[:, hh, qo:qo + P], mk_sb[:])
                        for hh in range(2):
                            h = 2 * hp + hh
                            nc.tensor.matmul(
                                ps_at[hh][:, qo:], v_sb[:, kt, h, :],
                                att[:, hh, qo:],
                                start=(kt == 0), stop=(kt == nkt - 1))
                        acc += rate
                        while acc >= 1.0 and steps:
                            steps.pop(0)()
                            acc -= 1.0
                    for hh in range(2):
                        norm_head(2 * hp + hh, c, ps_at[hh])
                # flush remaining filler for this chunk
                for f in steps:
                    f()

            for f in wo_steps(3):
                f()

    nc.compile()
    return nc


def _get_nc():
    if "nc" not in _CACHE:
        _CACHE["nc"] = _build()
    return _CACHE["nc"]


def _mask_const():
    # triangular 128x128: mask[k, t] = 1.0 iff t >= k
    t = np.arange(P)[None, :]
    k = np.arange(P)[:, None]
    return (t >= k).astype(ml_dtypes.bfloat16)


def _tile_xt(x_t):
    # [D_MODEL, S] -> [128, NCH, IT, 512]: the per-partition SBUF image
    return np.ascontiguousarray(
        x_t.reshape(IT, P, NCH, SCH).transpose(1, 2, 0, 3))


def _tile_w(w, blocks):
    # [(blocks*128), N] -> [128, blocks, N]: the per-partition SBUF image
    n = w.shape[1]
    return np.ascontiguousarray(w.reshape(blocks, P, n).transpose(1, 0, 2))


def _kernel_numpy(query, key, value, mask, Wq, Wk, Wv, Wo, bo):
    # exact f32 fallback for non-causal masks
    q = (query @ Wq.T).reshape(B, S, NUM_HEADS, HEAD_DIM).transpose(0, 2, 1, 3)
    k = (key @ Wk.T).reshape(B, S, NUM_HEADS, HEAD_DIM).transpose(0, 2, 1, 3)
    v = (value @ Wv.T).reshape(B, S, NUM_HEADS, HEAD_DIM).transpose(0, 2, 1, 3)
    s = np.einsum("bhqd,bhkd->bhqk", q, k) / np.sqrt(np.float32(HEAD_DIM))
    s = np.where(np.asarray(mask), s, -np.inf)
    s = s - s.max(axis=-1, keepdims=True)
    e = np.exp(s)
    a = e / e.sum(axis=-1, keepdims=True)
    o = np.einsum("bhqk,bhkd->bhqd", a, v).transpose(0, 2, 1, 3)
    return (o.reshape(B, S, D_MODEL) @ Wo.T + bo).astype(np.float32)


def kernel(query, key, value, mask, Wq, Wk, Wv, Wo, bo):
    from concourse.bass_utils import run_bass_kernel_spmd

    m = np.asarray(mask).astype(bool)
    expect = np.tril(np.ones((S, S), dtype=bool))
    if m.size != S * S or not np.array_equal(m.reshape(S, S), expect):
        args = [np.asarray(a, np.float32) for a in
                (query, key, value)] + [mask] + [
                np.asarray(a, np.float32) for a in (Wq, Wk, Wv, Wo, bo)]
        return _kernel_numpy(*args)

    nc = _get_nc()
    bf = ml_dtypes.bfloat16

    xq_t = [_tile_xt(np.asarray(query)[b].T.astype(bf)) for b in range(B)]
    xk_t = [_tile_xt(np.asarray(key)[b].T.astype(bf)) for b in range(B)]
    xv_t = [_tile_xt(np.asarray(value)[b].T.astype(bf)) for b in range(B)]
    WqT = np.ascontiguousarray(np.asarray(Wq).T).astype(bf)  # [D, D] cols = out dim
    WkT = np.ascontiguousarray(np.asarray(Wk).T).astype(bf)
    WvT = np.ascontiguousarray(np.asarray(Wv).T).astype(bf)
    WoT = np.ascontiguousarray(np.asarray(Wo).T).astype(bf)
    mk = _mask_const()

    in_maps = []
    for core in range(N_CORES):
        b, g = core // GROUPS, core % GROUPS
        hsl = slice(g * DLOC, (g + 1) * DLOC)
        in_maps.append({
            "xq_t": xq_t[b], "xk_t": xk_t[b], "xv_t": xv_t[b],
            "wq_t": _tile_w(WqT[:, hsl], IT),
            "wk_t": _tile_w(WkT[:, hsl], IT),
            "wv_t": _tile_w(WvT[:, hsl], IT),
            "wo_t": _tile_w(WoT[hsl, :], MB),
            "mask": mk,
        })

    res = run_bass_kernel_spmd(nc, in_maps, core_ids=list(range(N_CORES)))
    _CACHE["last_result"] = res

    out = np.zeros((B, S, D_MODEL), np.float32)
    for core in range(N_CORES):
        out[core // GROUPS] += np.asarray(res.results[core]["outp"],
                                          dtype=np.float32)
    out += np.asarray(bo, np.float32)[None, None, :]
    return out
